# revision 13
# baseline (speedup 1.0000x reference)
"""Trainium2 Bass kernel for CombinedEmbedding.

reference: out[b,s,f] = W @ x[b,s,f] + pos_emb[s] + fmap_emb[f],
with x a one-hot [B,S,F,V] float32 tensor.

Strategy (8 NeuronCores, data-parallel over tokens):
  - x's fp32 one-hot is transported as the high byte of each float
    (0x3F = 1.875 in fp8-e4m3, 0.0 stays 0) -- a lossless 4x shrink of
    the dominant HBM stream (128 MB -> 32 MB per core).  The host also
    transposes each core's slice to [V, 2048] so the vocab dim lands on
    SBUF partitions.
  - TensorE scans x: per 128-wide v-chunk, one matmul against a
    sliding-window fp8 stationary whose 3 live columns are
    (ones, j>>4, j&15) -- all e4m3-exact -- accumulates the chunk's
    (presence, m, r) digit rows into a PSUM bank; 32 chunks share a
    bank via zero columns.  128 chunks x 4 token-groups = 512 matmuls.
  - Decode: drains rescale the digit tables by 1/1.875 into exact bf16;
    small all-bf16 matmuls (table as stationary, exact integer decode
    weights as rhs) accumulate idx = 128*chunk + 16*m + r per token in
    [128,1] PSUM columns, summed across fills in SBUF.
  - The last fill runs token-group-major so each group's decode ->
    gather -> add -> store chain overlaps the remaining scan matmuls.
  - indirect-DMA gathers the matching bf16 rows of W^T [V, E]; one DVE
    add applies the host-preadded pos+fmap table (pre-permuted to the
    on-chip token layout); DMA out as f32.
"""

import numpy as np
import ml_dtypes

B, S, F, V, E = 4, 64, 64, 16384, 512
NCORES = 8
TOKENS = B * S * F            # 16384
TPC = TOKENS // NCORES        # 2048 tokens per core
P = 128                       # partitions
NCH = V // P                  # 128 v-chunks
NFILL = 4                     # psum fills per token-group sweep
CPF = NCH // NFILL            # 32 chunks per fill
NG = 4                        # token groups of 512
GTOK = TPC // NG              # 512
NSUB = 4                      # 128-token subgroups per group
DMA_B = 4                     # v-chunks per x DMA (1 MB transfers)
TPF = CPF // DMA_B            # 8 x-tiles per fill

FP8_ONE = 1.875               # value of byte 0x3F as e4m3

_cache = {}


def _build():
    import concourse.bass as bass
    import concourse.bacc as bacc
    import concourse.mybir as mybir
    import concourse.tile as tile
    from concourse.alu_op_type import AluOpType

    fp8 = mybir.dt.float8e4
    bf16 = mybir.dt.bfloat16
    f32 = mybir.dt.float32
    SCALE = 1.0 / FP8_ONE

    nc = bacc.Bacc(trn_type="TRN2")
    xT = nc.declare_dram_parameter("xT", [V, TPC], fp8, isOutput=False)
    wt = nc.declare_dram_parameter("wt", [V, E], bf16, isOutput=False)
    combo = nc.declare_dram_parameter("combo", [P, NG * NSUB, E], bf16,
                                      isOutput=False)
    stat = nc.declare_dram_parameter("stat", [P, 252], fp8, isOutput=False)
    wdec = nc.declare_dram_parameter("wdec", [P, NFILL], bf16, isOutput=False)
    out = nc.declare_dram_parameter("out", [P, NG * NSUB, E], f32,
                                    isOutput=True)

    xT_r = xT.rearrange("(nb four p) t -> nb p four t", four=DMA_B, p=P)
    wt_flat = wt[:, :]

    rings = [nc.sync, nc.scalar]
    xrings = [nc.sync, nc.scalar, nc.gpsimd]

    with tile.TileContext(nc) as tc:
        with (
            tc.tile_pool(name="pconst", bufs=1) as pconst,
            tc.tile_pool(name="px", bufs=14) as px,
            tc.tile_pool(name="ptab", bufs=2) as ptab,
            tc.tile_pool(name="pio", bufs=2) as pio,
            tc.tile_pool(name="pscan", bufs=2, space="PSUM") as pscan,
            tc.tile_pool(name="pdec", bufs=1, space="PSUM") as pdec,
        ):
            stat_sb = pconst.tile([P, 252], fp8)
            nc.sync.dma_start(out=stat_sb[:, :], in_=stat[:, :])
            wdec_sb = pconst.tile([P, NFILL], bf16)
            nc.sync.dma_start(out=wdec_sb[:, :], in_=wdec[:, :])
            comb_sb = pconst.tile([P, NG * NSUB, E], bf16)
            nc.scalar.dma_start(out=comb_sb[:, :, :], in_=combo[:, :, :])
            zbuf = pconst.tile([P, P], bf16)
            nc.vector.memset(zbuf[:, :], 0.0)
            idx_sb = pconst.tile([P, NG * NSUB], mybir.dt.int32)
            acc_sb = pconst.tile([P, NG * NSUB], f32)

            tabs = {}
            drain_k = [0]

            def drain(f, g):
                # psum digit bank -> exact bf16 table, rescaled by 1/1.875
                tab = ptab.tile([P, GTOK], bf16, tag=f"tab{g}", name="tab")
                tabs[(f, g)] = tab
                if drain_k[0] % 2 == 0:
                    nc.scalar.mul(out=tab[:, :], in_=banks[g][:, :], mul=SCALE)
                else:
                    nc.vector.tensor_scalar(
                        out=tab[:, :], in0=banks[g][:, :],
                        scalar1=SCALE, scalar2=None, op0=AluOpType.mult,
                    )
                drain_k[0] += 1

            def emit_dec(f, g):
                # decode fill f's table for group g: idx partial = weighted
                # sum over digit rows.  zero-matmul first so all 4 columns
                # have has_written set and the per-s matmuls accumulate.
                dec = pdec.tile([P, NSUB], f32, tag="dec", name="dec")
                nc.tensor.matmul(
                    dec[:, :], lhsT=zbuf[:, :], rhs=zbuf[:, 0:NSUB],
                    start=True, stop=False, skip_group_check=True,
                )
                tab = tabs.pop((f, g))
                for s in range(NSUB):
                    nc.tensor.matmul(
                        dec[:, s:s + 1],
                        lhsT=tab[:, s * P:(s + 1) * P],
                        rhs=wdec_sb[:, f:f + 1],
                        start=False,
                        stop=(s == NSUB - 1),
                        skip_group_check=True,
                    )
                if f == 0:
                    nc.vector.tensor_copy(acc_sb[:, 4 * g:4 * g + 4], dec[:, :])
                else:
                    nc.vector.tensor_tensor(
                        out=acc_sb[:, 4 * g:4 * g + 4],
                        in0=dec[:, :],
                        in1=acc_sb[:, 4 * g:4 * g + 4],
                        op=AluOpType.add,
                    )

            def finish_group(g):
                # per-subgroup idx -> gather -> +combo -> store chains so the
                # engines pipeline across subgroups
                gath = pio.tile([P, NSUB, E], bf16, tag="gath")
                outt = pio.tile([P, NSUB, E], f32, tag="out")
                for s in range(NSUB):
                    col = 4 * g + s
                    nc.vector.tensor_scalar(
                        out=idx_sb[:, col:col + 1],
                        in0=acc_sb[:, col:col + 1],
                        scalar1=0.25, scalar2=None, op0=AluOpType.add,
                    )
                    nc.gpsimd.indirect_dma_start(
                        out=gath[:, s, :],
                        out_offset=None,
                        in_=wt_flat,
                        in_offset=bass.IndirectOffsetOnAxis(
                            ap=idx_sb[:, col:col + 1], axis=0
                        ),
                    )
                    nc.vector.tensor_tensor(
                        out=outt[:, s, :],
                        in0=gath[:, s, :],
                        in1=comb_sb[:, col, :],
                        op=AluOpType.add,
                    )
                    rings[(g + s) % 2].dma_start(
                        out=out[:, col:col + 1, :], in_=outt[:, s:s + 1, :]
                    )

            def mm_chunk(bank, xt, j, cp, g):
                lhsT = stat_sb[:, 124 - 4 * cp:252 - 4 * cp]
                nc.tensor.matmul(
                    bank[:, :],
                    lhsT=lhsT,
                    rhs=xt[:, j, g * GTOK:(g + 1) * GTOK],
                    start=(cp == 0),
                    stop=(cp == CPF - 1),
                    skip_group_check=True,
                )

            # fills 0..2: chunk-major (stream tiles, all groups per chunk)
            for f in range(NFILL - 1):
                banks = []
                for g in range(NG):
                    bk = pscan.tile(
                        [P, GTOK], f32, tag=f"scan{g}", name=f"scan{g}",
                        bufs=(1 if g == NG - 1 else 2),
                    )
                    banks.append(bk)
                for dt_ in range(TPF):
                    xt = px.tile([P, DMA_B, TPC], fp8, tag="x")
                    xrings[(f * TPF + dt_) % 3].dma_start(
                        out=xt[:, :, :], in_=xT_r[f * TPF + dt_]
                    )
                    for j in range(DMA_B):
                        cp = dt_ * DMA_B + j
                        for g in range(NG):
                            mm_chunk(banks[g], xt, j, cp, g)
                    if dt_ == 0 and f > 0:
                        for g in range(NG):
                            emit_dec(f - 1, g)
                for g in [NG - 1] + list(range(NG - 1)):
                    drain(f, g)

            # last fill: group-major so each group's tail chain overlaps the
            # remaining scan matmuls
            f = NFILL - 1
            banks = []
            for g in range(NG):
                bk = pscan.tile(
                    [P, GTOK], f32, tag=f"scan{g}", name=f"scan{g}",
                    bufs=(1 if g == NG - 1 else 2),
                )
                banks.append(bk)
            tiles3 = []
            for dt_ in range(TPF):
                xt = px.tile([P, DMA_B, TPC], fp8, tag="x")
                xrings[(f * TPF + dt_) % 3].dma_start(
                    out=xt[:, :, :], in_=xT_r[f * TPF + dt_]
                )
                tiles3.append(xt)
            for g in range(NG):
                for dt_ in range(TPF):
                    for j in range(DMA_B):
                        mm_chunk(banks[g], tiles3[dt_], j, dt_ * DMA_B + j, g)
                if g == 0:
                    for gg in range(NG):
                        emit_dec(f - 1, gg)
                drain(f, g)
                emit_dec(f, g)
                finish_group(g)
    nc.finalize()
    return nc


def _prep_xt(x_flat):
    """[TOKENS, V] f32 one-hot -> per-core [V, TPC] fp8-e4m3 byte views."""
    try:
        import jax
        import jax.numpy as jnp
        cpu = jax.devices("cpu")[0]

        def _f(xc):
            u = jax.lax.bitcast_convert_type(xc, jnp.uint32)
            return (u >> 24).astype(jnp.uint8).T

        jf = jax.jit(_f)
        outs = []
        with jax.default_device(cpu):
            for c in range(NCORES):
                a = np.asarray(jf(x_flat[c * TPC:(c + 1) * TPC]))
                outs.append(a.view(ml_dtypes.float8_e4m3))
        return outs
    except Exception:
        xb = x_flat.view(np.uint8).reshape(TOKENS, V, 4)[:, :, 3]
        outs = []
        for c in range(NCORES):
            a = np.ascontiguousarray(xb[c * TPC:(c + 1) * TPC])
            aT = np.empty((V, TPC), np.uint8)
            for i0 in range(0, TPC, 128):
                aT[:, i0:i0 + 128] = a[i0:i0 + 128, :].T
            outs.append(aT.view(ml_dtypes.float8_e4m3))
        return outs


def _host_shards(x, W, pos_emb, fmap_emb):
    x_flat = np.ascontiguousarray(x.reshape(TOKENS, V))
    xts = _prep_xt(x_flat)
    wt = np.ascontiguousarray(W.T).astype(ml_dtypes.bfloat16)    # [V, E]
    fmap_t = np.tile(fmap_emb[:F], (TPC // F, 1))                # [2048, E]

    # stationary: rows j, cols 124..126 = (1, j>>4, j&15); window slide
    # stat[:, 124-4c : 252-4c] puts them at output partitions 4c+0..2
    st = np.zeros((P, 252), np.float32)
    st[:, 124] = 1.0
    st[:, 125] = np.arange(P) >> 4
    st[:, 126] = np.arange(P) & 15
    stat = st.astype(ml_dtypes.float8_e4m3)

    # decode weights (exact in bf16): psum row 4c+d of fill f -> idx weight
    rows = np.arange(P)
    cpr, d = rows // 4, rows % 4
    wdec = np.zeros((P, NFILL), np.float32)
    for f in range(NFILL):
        wdec[:, f] = np.select(
            [d == 0, d == 1, d == 2],
            [128.0 * (CPF * f + cpr), 16.0, 1.0], 0.0,
        )
    wdec = wdec.astype(ml_dtypes.bfloat16)

    in_maps = []
    for c in range(NCORES):
        s_base = (c % 2) * 32
        pos2 = np.repeat(pos_emb[s_base:s_base + TPC // F], F, axis=0)
        combo = (pos2 + fmap_t).astype(ml_dtypes.bfloat16)       # [2048, E]
        # permute to the on-chip token layout: [p, (g,s), e]
        combo_p = np.ascontiguousarray(
            combo.reshape(NG * NSUB, P, E).transpose(1, 0, 2)
        )
        in_maps.append({
            "xT": xts[c],
            "wt": wt,
            "combo": combo_p,
            "stat": stat,
            "wdec": wdec,
        })
    return in_maps


def kernel(x, W, pos_emb, fmap_emb):
    from concourse import bass_utils

    x = np.asarray(x, dtype=np.float32)
    W = np.asarray(W, dtype=np.float32)
    pos_emb = np.asarray(pos_emb, dtype=np.float32)
    fmap_emb = np.asarray(fmap_emb, dtype=np.float32)

    if "nc" not in _cache:
        _cache["nc"] = _build()
    nc = _cache["nc"]

    in_maps = _host_shards(x, W, pos_emb, fmap_emb)
    res = bass_utils.run_bass_kernel_spmd(nc, in_maps, core_ids=list(range(NCORES)))
    outs = []
    for c in range(NCORES):
        o = res.results[c]["out"]                     # [P, 16, E]
        outs.append(np.transpose(o, (1, 0, 2)).reshape(TPC, E))
    full = np.concatenate(outs, axis=0).reshape(B, S, F, E)
    return full
